# revision 6
# baseline (speedup 1.0000x reference)
"""TGNMemory update kernel for 8 Trainium2 NeuronCores.

Strategy (graph-partitioned, per sharding hint):
  - Nodes are sharded contiguously across the 8 cores (25000 nodes each).
  - Host routes each event to the core owning its destination node and
    resolves the LastAggregator winner per node (cheap index work -- the
    "all-to-all" collapses into host-side staging since I/O is full).
  - Each core's nodes are permuted so nodes with a winning event come
    first; packed per-winner operands (other-endpoint memory row, raw
    message, time-encoder argument) are staged in feature-major layout.
  - The device does all the heavy math: time encoding (ACT Sin), the
    GRU gate matmuls (PE, fp32, PSUM K-chunk accumulation), gate
    nonlinearities (ACT) and the blend (DVE), with contiguous DMA only.
  - Output is new_memory (device) + new_last_update (host index math).

GRU algebra used on device (feature-major, gates on partitions):
  r = sigmoid(W_r_mem@mem + W_r_oth@oth + W_r_raw@raw + W_r_enc@enc + br)
      where W_r_mem = w_ih_r[:, :100] + w_hh_r   (both multiply mem[n])
  z = likewise
  n = tanh(gi_n + b_ih_n + r * (gh_n + b_hh_n))
  out = n + z * (mem - n)
Nodes without events: gi = b_ih only (dense pass).
"""

import os
import sys

import numpy as np

for _p in ("/opt/trn_rl_repo", "/root/.axon_site/_ro/trn_rl_repo"):
    if os.path.isdir(_p) and _p not in sys.path:
        sys.path.append(_p)

N = 200000
E = 100000
RAW = 172
MEM = 100
TDIM = 100
H = 100
NCORES = 8
NPC = N // NCORES  # nodes per core
TILE = 512

LAST_RESULTS = None  # BassKernelResults of the most recent run (for profiling)

_PROGRAM_CACHE = {}


def _round_up(x, m):
    return ((x + m - 1) // m) * m


def build_program(a_pad, d_start, repeat=1):
    """Build the SPMD Bass program (same for all 8 cores).

    a_pad: padded packed-region length (multiple of TILE, >= every core's
           active count). d_start: first node position of the dense pass
           (multiple of TILE, <= every core's active count).
    repeat: benchmarking aid -- run the whole body `repeat` times inside a
            device-side loop so wall-clock deltas isolate device time.
    """
    import concourse.mybir as mybir
    from concourse import bacc, tile

    f32 = mybir.dt.float32
    AF = mybir.ActivationFunctionType
    OP = mybir.AluOpType

    nc = bacc.Bacc(
        "TRN2",
        target_bir_lowering=False,
        debug=False,
        enable_asserts=False,
        num_devices=NCORES,
    )

    mem_T = nc.dram_tensor("mem_T", [MEM, NPC], f32, kind="ExternalInput").ap()
    other_T = nc.dram_tensor("other_T", [MEM, a_pad], f32, kind="ExternalInput").ap()
    raw_T = nc.dram_tensor("raw_T", [RAW, a_pad], f32, kind="ExternalInput").ap()
    arg_T = nc.dram_tensor("arg_T", [TDIM, a_pad], f32, kind="ExternalInput").ap()
    # w100 columns: [0:300] mem-chunk weights (r,z,n)  (r/z have w_hh folded in)
    #              [300:600] other-chunk, [600:900] enc-chunk,
    #              [900:1100] dense w_hh r/z, [1100:1200] w_hh n
    w100 = nc.dram_tensor("w100", [100, 1200], f32, kind="ExternalInput").ap()
    w128 = nc.dram_tensor("w128", [128, 300], f32, kind="ExternalInput").ap()
    w44 = nc.dram_tensor("w44", [44, 300], f32, kind="ExternalInput").ap()
    # consts columns: 0 bias_r (b_ih+b_hh), 1 bias_z, 2 b_hh_n, 3 b_ih_n
    consts = nc.dram_tensor("consts", [100, 4], f32, kind="ExternalInput").ap()

    outp_T = nc.dram_tensor("outp_T", [MEM, a_pad], f32, kind="ExternalOutput").ap()
    outd_T = nc.dram_tensor(
        "outd_T", [MEM, NPC - d_start], f32, kind="ExternalOutput"
    ).ap()

    with tile.TileContext(nc) as tc:
        with (
            tc.tile_pool(name="wpool", bufs=1) as wpool,
            tc.tile_pool(name="inp", bufs=3) as inp,
            tc.tile_pool(name="tmp", bufs=3) as tmp,
            tc.tile_pool(name="outp", bufs=3) as outp,
            tc.tile_pool(name="psum", bufs=2, space="PSUM") as psum,
        ):
            w100_t = wpool.tile([100, 1200], f32)
            nc.sync.dma_start(w100_t[:], w100[:])
            w128_t = wpool.tile([128, 300], f32)
            nc.sync.dma_start(w128_t[:], w128[:])
            w44_t = wpool.tile([44, 300], f32)
            nc.sync.dma_start(w44_t[:], w44[:])
            cst = wpool.tile([100, 4], f32)
            nc.sync.dma_start(cst[:], consts[:])

            def gates_tail(pr, pz, pn_in, pg, mem_tile, w, out_ap):
                """sigmoids, tanh, blend, and the output DMA for one tile."""
                r = tmp.tile([100, w], f32, tag="r")
                nc.scalar.activation(r[:], pr[:], AF.Sigmoid, bias=cst[:, 0:1])
                z = tmp.tile([100, w], f32, tag="z")
                nc.scalar.activation(z[:], pz[:], AF.Sigmoid, bias=cst[:, 1:2])
                # t1 = (gh_n + b_hh_n) * r
                t1 = tmp.tile([100, w], f32, tag="t1")
                nc.vector.scalar_tensor_tensor(
                    t1[:], pg[:], cst[:, 2:3], r[:], op0=OP.add, op1=OP.mult
                )
                if pn_in is not None:
                    t2 = tmp.tile([100, w], f32, tag="t2")
                    nc.vector.tensor_add(t2[:], t1[:], pn_in[:])
                else:
                    t2 = t1
                ng = tmp.tile([100, w], f32, tag="ng")
                nc.scalar.activation(ng[:], t2[:], AF.Tanh, bias=cst[:, 3:4])
                d = tmp.tile([100, w], f32, tag="d")
                nc.vector.tensor_sub(d[:], mem_tile[:, :w], ng[:])
                ee = tmp.tile([100, w], f32, tag="ee")
                nc.vector.tensor_mul(ee[:], z[:], d[:])
                o = outp.tile([100, w], f32, tag="o")
                nc.vector.tensor_add(o[:], ng[:], ee[:])
                nc.sync.dma_start(out_ap, o[:])

            def packed_tile(i):
                lo = i * TILE
                sl = slice(lo, lo + TILE)
                c0 = inp.tile([100, TILE], f32, tag="c0")
                nc.sync.dma_start(c0[:], mem_T[:, sl])
                c1 = inp.tile([100, TILE], f32, tag="c1")
                nc.sync.dma_start(c1[:], other_T[:, sl])
                c2 = inp.tile([128, TILE], f32, tag="c2")
                nc.sync.dma_start(c2[:], raw_T[0:128, sl])
                c3 = inp.tile([44, TILE], f32, tag="c3")
                nc.sync.dma_start(c3[:], raw_T[128:172, sl])
                db = inp.tile([100, TILE], f32, tag="db")
                nc.sync.dma_start(db[:], arg_T[:, sl])

                enc = tmp.tile([100, TILE], f32, tag="enc")
                nc.scalar.activation(enc[:], db[:], AF.Sin)

                pr = psum.tile([100, TILE], f32, tag="pr")
                pz = psum.tile([100, TILE], f32, tag="pz")
                pn = psum.tile([100, TILE], f32, tag="pn")
                pg = psum.tile([100, TILE], f32, tag="pg")
                for p, oc in ((pr, 0), (pz, 100), (pn, 200)):
                    nc.tensor.matmul(
                        p[:], w100_t[:, oc : oc + 100], c0[:], start=True, stop=False
                    )
                    nc.tensor.matmul(
                        p[:],
                        w100_t[:, 300 + oc : 400 + oc],
                        c1[:],
                        start=False,
                        stop=False,
                    )
                    nc.tensor.matmul(
                        p[:], w128_t[:, oc : oc + 100], c2[:], start=False, stop=False
                    )
                    nc.tensor.matmul(
                        p[:], w44_t[:, oc : oc + 100], c3[:], start=False, stop=False
                    )
                    nc.tensor.matmul(
                        p[:],
                        w100_t[:, 600 + oc : 700 + oc],
                        enc[:],
                        start=False,
                        stop=True,
                    )
                nc.tensor.matmul(pg[:], w100_t[:, 1100:1200], c0[:])
                gates_tail(pr, pz, pn, pg, c0, TILE, outp_T[:, sl])

            def dense_tile(i):
                span = NPC - d_start
                lo = i * TILE
                w = min(TILE, span - lo)
                sl = slice(d_start + lo, d_start + lo + w)
                m = inp.tile([100, w], f32, tag="dm")
                nc.sync.dma_start(m[:], mem_T[:, sl])
                pr = psum.tile([100, w], f32, tag="pr")
                pz = psum.tile([100, w], f32, tag="pz")
                pg = psum.tile([100, w], f32, tag="pg")
                nc.tensor.matmul(pr[:], w100_t[:, 900:1000], m[:])
                nc.tensor.matmul(pz[:], w100_t[:, 1000:1100], m[:])
                nc.tensor.matmul(pg[:], w100_t[:, 1100:1200], m[:])
                gates_tail(pr, pz, None, pg, m, w, outd_T[:, lo : lo + w])

            def body():
                for i in range(a_pad // TILE):
                    packed_tile(i)
                for i in range(_round_up(NPC - d_start, TILE) // TILE):
                    dense_tile(i)

            if repeat > 1:
                with tc.For_i(0, repeat, 1):
                    body()
            else:
                body()

    nc.compile()
    return nc


def prepare(memory, raw_msg, time_w, time_b, w_ih, w_hh, b_ih, b_hh, last_update, src, dst, t):
    """Host-side routing/staging. Returns (a_pad, d_start, in_maps, meta)."""
    f32 = np.float32
    src = np.asarray(src)
    dst = np.asarray(dst)
    t = np.asarray(t)
    memory = np.asarray(memory, dtype=f32)
    raw_msg = np.asarray(raw_msg, dtype=f32)
    last_update = np.asarray(last_update)

    # ---- LastAggregator winner per node.
    # Entry j in [0,E) is the src-side copy of event j (destination src[j]);
    # entry E+j the dst-side copy (destination dst[j]).  The reference keeps,
    # per node, the entry maximizing (t, j).  lexsort is stable, so sorting
    # by (idx, t) leaves equal (idx, t) in ascending-j order; the last entry
    # of each idx-group is the winner.
    idx = np.concatenate([src, dst]).astype(np.int64)
    t2 = np.concatenate([t, t])
    order = np.lexsort((t2, idx))
    sidx = idx[order]
    last = np.empty(2 * E, dtype=bool)
    last[:-1] = sidx[1:] != sidx[:-1]
    last[-1] = True
    win_j = order[last]          # winning entry per active node
    win_node = sidx[last]        # active node ids, ascending
    ev = (win_j % E).astype(np.int64)
    dst_side = win_j >= E
    other = np.where(dst_side, src[ev], dst[ev]).astype(np.int64)
    max_t = t[ev]

    new_last_update = np.array(last_update, copy=True)
    new_last_update[win_node] = max_t

    core_of = win_node // NPC
    counts = np.bincount(core_of, minlength=NCORES)
    starts = np.concatenate([[0], np.cumsum(counts)])
    a_pad = max(TILE, _round_up(int(counts.max()), TILE))
    a_pad = min(a_pad, _round_up(NPC, TILE))
    d_start = (int(counts.min()) // TILE) * TILE

    # ---- weights (replicated)
    w_ih = np.asarray(w_ih, dtype=f32)
    w_hh = np.asarray(w_hh, dtype=f32)
    b_ih = np.asarray(b_ih, dtype=f32)
    b_hh = np.asarray(b_hh, dtype=f32)
    w100 = np.empty((100, 1200), dtype=f32)
    for g in range(3):  # r, z, n
        gs = slice(100 * g, 100 * g + 100)
        blk = w_ih[gs, 0:100].T.copy()
        if g < 2:
            blk += w_hh[gs, :].T  # mem[n] multiplies both w_ih[:, :100] and w_hh
        w100[:, 100 * g : 100 * g + 100] = blk
        w100[:, 300 + 100 * g : 400 + 100 * g] = w_ih[gs, 100:200].T
        w100[:, 600 + 100 * g : 700 + 100 * g] = w_ih[gs, 372:472].T
    w100[:, 900:1000] = w_hh[0:100, :].T
    w100[:, 1000:1100] = w_hh[100:200, :].T
    w100[:, 1100:1200] = w_hh[200:300, :].T
    w128 = np.ascontiguousarray(w_ih[:, 200:328].T)
    w44 = np.ascontiguousarray(w_ih[:, 328:372].T)
    consts = np.empty((100, 4), dtype=f32)
    consts[:, 0] = b_ih[0:100] + b_hh[0:100]
    consts[:, 1] = b_ih[100:200] + b_hh[100:200]
    consts[:, 2] = b_hh[200:300]
    consts[:, 3] = b_ih[200:300]

    tw = np.asarray(time_w, dtype=f32)[:, 0]
    tb = np.asarray(time_b, dtype=f32)

    in_maps = []
    meta = []
    lu_f = last_update.astype(f32)
    tf = t.astype(f32)
    for c in range(NCORES):
        s0, s1 = starts[c], starts[c + 1]
        na = s1 - s0
        wn = (win_node[s0:s1] - c * NPC).astype(np.int64)  # local ids, ascending
        oth = other[s0:s1]
        evc = ev[s0:s1]

        act = np.zeros(NPC, dtype=bool)
        act[wn] = True
        inact = np.nonzero(~act)[0]
        perm = np.concatenate([wn, inact])

        mem_T = np.empty((MEM, NPC), dtype=f32)
        mem_T[:, :] = memory[c * NPC + perm].T

        other_T = np.zeros((MEM, a_pad), dtype=f32)
        other_T[:, :na] = memory[oth].T

        raw_T = np.zeros((RAW, a_pad), dtype=f32)
        raw_T[:, :na] = raw_msg[evc].T

        # time-encoder argument: reference computes x = fl32(dt)*w + b then
        # cos(x).  We send y = range-reduced (x + pi/2) so the device's
        # ACT Sin sees [-pi, pi] and sin(y) == cos(x) to f32 precision.
        dtv = tf[evc] - lu_f[c * NPC + wn]  # f32, exact (ints < 2^24)
        x = dtv[:, None] * tw[None, :] + tb[None, :]  # f32 [na, 100]
        y = np.mod(x.astype(np.float64) + (np.pi / 2 + np.pi), 2 * np.pi) - np.pi
        arg_T = np.zeros((TDIM, a_pad), dtype=f32)
        arg_T[:, :na] = y.T.astype(f32)

        in_maps.append(
            dict(
                mem_T=mem_T,
                other_T=other_T,
                raw_T=raw_T,
                arg_T=arg_T,
                w100=w100,
                w128=w128,
                w44=w44,
                consts=consts,
            )
        )
        meta.append((na, wn, inact))

    return a_pad, d_start, in_maps, (meta, new_last_update)


def assemble(a_pad, d_start, results, meta_pack):
    meta, new_last_update = meta_pack
    new_memory = np.empty((N, MEM), dtype=np.float32)
    for c in range(NCORES):
        na, wn, inact = meta[c]
        outp = results[c]["outp_T"]  # [MEM, a_pad]
        outd = results[c]["outd_T"]  # [MEM, NPC - d_start]
        base = c * NPC
        new_memory[base + wn] = outp[:, :na].T
        new_memory[base + inact] = outd[:, na - d_start :].T
    return new_memory, new_last_update


def kernel(**inputs):
    global LAST_RESULTS
    a_pad, d_start, in_maps, meta_pack = prepare(**inputs)

    key = (a_pad, d_start)
    if key not in _PROGRAM_CACHE:
        _PROGRAM_CACHE[key] = build_program(a_pad, d_start)
    nc = _PROGRAM_CACHE[key]

    from concourse.bass_utils import run_bass_kernel_spmd

    res = run_bass_kernel_spmd(nc, in_maps, core_ids=list(range(NCORES)))
    LAST_RESULTS = res
    return assemble(a_pad, d_start, res.results, meta_pack)


# revision 23
# speedup vs baseline: 32.7091x; 32.7091x over previous
"""TGNMemory update kernel for 8 Trainium2 NeuronCores.

Strategy (graph-partitioned, per sharding hint):
  - Nodes are sharded contiguously across the 8 cores (25000 nodes each).
  - Host routes each event to the core owning its destination node and
    resolves the LastAggregator winner per node (cheap index work -- the
    "all-to-all" collapses into host-side staging since I/O is full).
  - Each core's nodes are permuted so nodes with a winning event come
    first; packed per-winner operands (other-endpoint memory row, raw
    message, time-encoder argument) are staged in feature-major layout.
  - The device does all the heavy math: time encoding (ACT Sin), the
    GRU gate matmuls (PE, fp32, PSUM K-chunk accumulation), gate
    nonlinearities (ACT) and the blend (DVE), with contiguous DMA only.
  - Output is new_memory (device) + new_last_update (host index math).

GRU algebra used on device (feature-major, gates on partitions):
  r = sigmoid(W_r_mem@mem + W_r_oth@oth + W_r_raw@raw + W_r_enc@enc + br)
      where W_r_mem = w_ih_r[:, :100] + w_hh_r   (both multiply mem[n])
  z = likewise
  n = tanh(gi_n + b_ih_n + r * (gh_n + b_hh_n))
  out = n + z * (mem - n)
Nodes without events: gi = b_ih only (dense pass).
"""

import os
import sys

import numpy as np

for _p in ("/opt/trn_rl_repo", "/root/.axon_site/_ro/trn_rl_repo"):
    if os.path.isdir(_p) and _p not in sys.path:
        sys.path.append(_p)

N = 200000
E = 100000
RAW = 172
MEM = 100
TDIM = 100
H = 100
NCORES = 8
NPC = N // NCORES  # nodes per core
TILE = 512
GROUP = 4  # tiles per input-DMA batch

LAST_RESULTS = None  # BassKernelResults of the most recent run (for profiling)

_PROGRAM_CACHE = {}


def _round_up(x, m):
    return ((x + m - 1) // m) * m


def build_program(a_pad, d_start, repeat=1):
    """Build the SPMD Bass program (same for all 8 cores).

    a_pad: padded packed-region length (multiple of TILE, >= every core's
           active count). d_start: first node position of the dense pass
           (multiple of TILE, <= every core's active count).
    repeat: benchmarking aid -- run the whole body `repeat` times inside a
            device-side loop so wall-clock deltas isolate device time.
    """
    import concourse.mybir as mybir
    from concourse import bacc, tile

    f32 = mybir.dt.float32
    AF = mybir.ActivationFunctionType
    OP = mybir.AluOpType

    nc = bacc.Bacc(
        "TRN2",
        target_bir_lowering=False,
        debug=False,
        enable_asserts=False,
        num_devices=NCORES,
    )

    f32r = mybir.dt.float32r
    f16 = mybir.dt.float16
    mem_T = nc.dram_tensor("mem_T", [MEM + 1, NPC], f32r, kind="ExternalInput").ap()
    other_T = nc.dram_tensor("other_T", [MEM, a_pad], f16, kind="ExternalInput").ap()
    raw_T = nc.dram_tensor("raw_T", [RAW, a_pad], f16, kind="ExternalInput").ap()
    arg_T = nc.dram_tensor("arg_T", [TDIM, a_pad], f32, kind="ExternalInput").ap()
    # wA (f32r) columns: [0:300] mem-chunk r/z/n (r,z have w_hh folded in),
    #                    [300:500] dense w_hh r/z, [500:600] w_hh n
    # row 100 is the all-ones bias row: biases ride the matmul
    wA = nc.dram_tensor("wA", [101, 600], f32r, kind="ExternalInput").ap()
    # wB (bf16) columns: [0:300] other r/z/n (rows 0:100),
    #   [300:600] raw[0:128] (rows 0:128), [600:900] raw[128:172] (rows 0:44),
    #   [900:1200] enc (rows 0:100)
    wB = nc.dram_tensor("wB", [128, 1200], f16, kind="ExternalInput").ap()
    # consts columns: 0 bias_r (b_ih+b_hh), 1 bias_z, 2 b_hh_n, 3 b_ih_n
    consts = nc.dram_tensor("consts", [100, 4], f32, kind="ExternalInput").ap()

    outp_T = nc.dram_tensor("outp_T", [MEM, a_pad], f32, kind="ExternalOutput").ap()
    outd_T = nc.dram_tensor(
        "outd_T", [MEM, NPC - d_start], f32, kind="ExternalOutput"
    ).ap()

    GRP = GROUP * TILE  # input-DMA batch width

    with tile.TileContext(nc) as tc:
        with (
            tc.tile_pool(name="wpool", bufs=1) as wpool,
            tc.tile_pool(name="inp", bufs=2) as inp,
            tc.tile_pool(name="encp", bufs=2) as encp,
            tc.tile_pool(name="tmp", bufs=3) as tmp,
            tc.tile_pool(name="outp", bufs=2) as outp,
            tc.tile_pool(name="psum", bufs=2, space="PSUM") as psum,
        ):
            wA_t = wpool.tile([101, 600], f32r)
            nc.sync.dma_start(wA_t[:], wA[:])
            wB_t = wpool.tile([128, 1200], f16)
            nc.sync.dma_start(wB_t[:], wB[:])
            cst = wpool.tile([100, 4], f32)
            nc.sync.dma_start(cst[:], consts[:])

            def mm(p, lhsT, rhs, start, stop):
                # float32r operands: same bits as f32, 4x PE throughput
                nc.tensor.matmul(p, lhsT, rhs, start=start, stop=stop)

            def gates_tail(pr, pz, pn_in, pg, mem_ap, w, ob, ocol):
                """sigmoids, tanh, blend into ob[:, ocol:ocol+w].

                All gate biases ride the matmuls via the ones row of the mem
                chunk, so the ACT ops carry no bias reads."""
                # sigmoid via tanh identity: sigma(x) = (1 + tanh(x/2)) / 2.
                # tanh and sin share one ACT func table (silu_and_others), so
                # the whole kernel runs with zero table reloads.  The +1 and
                # x0.5 fold into the downstream scalar_tensor_tensor ops.
                rt = tmp.tile([100, TILE], f32, tag="rt")
                nc.scalar.activation(rt[:, :w], pr[:, :w], AF.Tanh, scale=0.5)
                zt = tmp.tile([100, TILE], f32, tag="zt")
                nc.scalar.activation(zt[:, :w], pz[:, :w], AF.Tanh, scale=0.5)
                # u1 = (tanh_r + 1) * (gh_n + b_hh_n) = 2 r * pg
                u1 = tmp.tile([100, TILE], f32, tag="u1")
                nc.vector.scalar_tensor_tensor(
                    u1[:, :w],
                    rt[:, :w],
                    1.0,
                    pg[:, :w] if pn_in is None else pg[:],
                    op0=OP.add,
                    op1=OP.mult,
                )
                ng = tmp.tile([100, TILE], f32, tag="ng")
                if pn_in is not None:
                    # t2 = 0.5 u1 + gi_n
                    t2 = tmp.tile([100, TILE], f32, tag="t2")
                    nc.vector.scalar_tensor_tensor(
                        t2[:, :w], u1[:, :w], 0.5, pn_in[:], op0=OP.mult, op1=OP.add
                    )
                    nc.scalar.activation(ng[:, :w], t2[:, :w], AF.Tanh)
                else:
                    nc.scalar.activation(
                        ng[:, :w], u1[:, :w], AF.Tanh, bias=cst[:, 3:4], scale=0.5
                    )
                # blend: ob = ng + z (mem - ng) with z = (tanh_z + 1) / 2
                d = tmp.tile([100, TILE], f32, tag="d")
                nc.gpsimd.tensor_sub(d[:, :w], mem_ap.bitcast(f32), ng[:, :w])
                ee = tmp.tile([100, TILE], f32, tag="ee")
                nc.vector.scalar_tensor_tensor(
                    ee[:, :w], zt[:, :w], 1.0, d[:, :w], op0=OP.add, op1=OP.mult
                )
                nc.vector.scalar_tensor_tensor(
                    ob[:, ocol : ocol + w], ee[:, :w], 0.5, ng[:, :w],
                    op0=OP.mult, op1=OP.add,
                )

            def packed_group(g, enc, eoff):
                glo = g * GRP
                gw = min(GRP, a_pad - glo)
                sl = slice(glo, glo + gw)
                c0 = inp.tile([101, GRP], f32r, tag="c0")
                nc.sync.dma_start(c0[:, :gw], mem_T[:, sl])
                c1 = inp.tile([100, GRP], f16, tag="c1")
                nc.sync.dma_start(c1[:, :gw], other_T[:, sl])
                c2 = inp.tile([128, GRP], f16, tag="c2")
                nc.sync.dma_start(c2[:, :gw], raw_T[0:128, sl])
                c3 = inp.tile([44, GRP], f16, tag="c3")
                nc.sync.dma_start(c3[:, :gw], raw_T[128:172, sl])
                ob = outp.tile([100, GRP], f32, tag="ob")

                for j in range(gw // TILE):
                    cs = slice(j * TILE, (j + 1) * TILE)
                    es = slice(eoff + j * TILE, eoff + (j + 1) * TILE)
                    pr = psum.tile([100, TILE], f32, tag="pr")
                    pz = psum.tile([100, TILE], f32, tag="pz")
                    pn = psum.tile([100, TILE], f32, tag="pn")
                    pg = psum.tile([100, TILE], f32, tag="pg")
                    for p, oc in ((pr, 0), (pz, 100), (pn, 200)):
                        mm(p[:], wA_t[:, oc : oc + 100], c0[:, cs], True, False)
                        mm(p[:], wB_t[0:100, oc : oc + 100], c1[:, cs], False, False)
                        mm(p[:], wB_t[0:128, 300 + oc : 400 + oc], c2[:, cs], False, False)
                        mm(p[:], wB_t[0:44, 600 + oc : 700 + oc], c3[:, cs], False, False)
                        mm(p[:], wB_t[0:100, 900 + oc : 1000 + oc], enc[:, es], False, True)
                    mm(pg[:], wA_t[:, 500:600], c0[:, cs], True, True)
                    gates_tail(pr, pz, pn, pg, c0[0:100, cs], TILE, ob, j * TILE)
                nc.sync.dma_start(outp_T[:, sl], ob[:, :gw])

            def dense_group(g):
                span = NPC - d_start
                glo = g * GRP
                gw = min(GRP, span - glo)
                sl = slice(d_start + glo, d_start + glo + gw)
                m = inp.tile([101, GRP], f32r, tag="c0")
                nc.sync.dma_start(m[:, :gw], mem_T[:, sl])
                ob = outp.tile([100, GRP], f32, tag="ob")
                for j in range(_round_up(gw, TILE) // TILE):
                    w = min(TILE, gw - j * TILE)
                    cs = slice(j * TILE, j * TILE + w)
                    pr = psum.tile([100, TILE], f32, tag="pr")
                    pz = psum.tile([100, TILE], f32, tag="pz")
                    pg = psum.tile([100, TILE], f32, tag="pg")
                    mm(pr[:, :w], wA_t[:, 300:400], m[:, cs], True, True)
                    mm(pz[:, :w], wA_t[:, 400:500], m[:, cs], True, True)
                    mm(pg[:, :w], wA_t[:, 500:600], m[:, cs], True, True)
                    gates_tail(pr, pz, None, pg[:, :w], m[0:100, cs], w, ob, j * TILE)
                nc.sync.dma_start(outd_T[:, glo : glo + gw], ob[:, :gw])

            def body():
                ng = _round_up(a_pad, GRP) // GRP
                # Sin shares no ACT func-table set with sigmoid/tanh, so one
                # Sin per pair of groups halves the table reloads
                for pp in range(_round_up(ng, 2) // 2):
                    plo = pp * 2 * GRP
                    pw = min(2 * GRP, a_pad - plo)
                    db = inp.tile([100, 2 * GRP], f32, tag="db")
                    nc.sync.dma_start(db[:, :pw], arg_T[:, plo : plo + pw])
                    enc = encp.tile([100, 2 * GRP], f16, tag="enc")
                    nc.scalar.activation(enc[:, :pw], db[:, :pw], AF.Sin)
                    for gg in range(2):
                        g = pp * 2 + gg
                        if g < ng:
                            packed_group(g, enc, gg * GRP)
                for g in range(_round_up(NPC - d_start, GRP) // GRP):
                    dense_group(g)

            if repeat > 1:
                with tc.For_i(0, repeat, 1):
                    body()
            else:
                body()

    nc.compile()
    return nc


def prepare(memory, raw_msg, time_w, time_b, w_ih, w_hh, b_ih, b_hh, last_update, src, dst, t):
    """Host-side routing/staging. Returns (a_pad, d_start, in_maps, meta)."""
    f32 = np.float32
    src = np.asarray(src)
    dst = np.asarray(dst)
    t = np.asarray(t)
    memory = np.asarray(memory, dtype=f32)
    raw_msg = np.asarray(raw_msg, dtype=f32)
    last_update = np.asarray(last_update)

    # ---- LastAggregator winner per node.
    # Entry j in [0,E) is the src-side copy of event j (destination src[j]);
    # entry E+j the dst-side copy (destination dst[j]).  The reference keeps,
    # per node, the entry maximizing (t, j).  lexsort is stable, so sorting
    # by (idx, t) leaves equal (idx, t) in ascending-j order; the last entry
    # of each idx-group is the winner.
    idx = np.concatenate([src, dst]).astype(np.int64)
    t2 = np.concatenate([t, t])
    order = np.lexsort((t2, idx))
    sidx = idx[order]
    last = np.empty(2 * E, dtype=bool)
    last[:-1] = sidx[1:] != sidx[:-1]
    last[-1] = True
    win_j = order[last]          # winning entry per active node
    win_node = sidx[last]        # active node ids, ascending
    ev = (win_j % E).astype(np.int64)
    dst_side = win_j >= E
    other = np.where(dst_side, src[ev], dst[ev]).astype(np.int64)
    max_t = t[ev]

    new_last_update = np.array(last_update, copy=True)
    new_last_update[win_node] = max_t

    core_of = win_node // NPC
    counts = np.bincount(core_of, minlength=NCORES)
    starts = np.concatenate([[0], np.cumsum(counts)])
    a_pad = max(TILE, _round_up(int(counts.max()), TILE))
    a_pad = min(a_pad, _round_up(NPC, TILE))
    d_start = (int(counts.min()) // TILE) * TILE

    # ---- weights (replicated)
    w_ih = np.asarray(w_ih, dtype=f32)
    w_hh = np.asarray(w_hh, dtype=f32)
    b_ih = np.asarray(b_ih, dtype=f32)
    b_hh = np.asarray(b_hh, dtype=f32)
    wA = np.zeros((101, 600), dtype=f32)
    wB = np.zeros((128, 1200), dtype=np.float16)
    for g in range(3):  # r, z, n
        gs = slice(100 * g, 100 * g + 100)
        blk = w_ih[gs, 0:100].T.copy()
        if g < 2:
            blk += w_hh[gs, :].T  # mem[n] multiplies both w_ih[:, :100] and w_hh
        wA[0:100, 100 * g : 100 * g + 100] = blk
        wB[0:100, 100 * g : 100 * g + 100] = w_ih[gs, 100:200].T
        wB[0:128, 300 + 100 * g : 400 + 100 * g] = w_ih[gs, 200:328].T
        wB[0:44, 600 + 100 * g : 700 + 100 * g] = w_ih[gs, 328:372].T
        wB[0:100, 900 + 100 * g : 1000 + 100 * g] = w_ih[gs, 372:472].T
    wA[0:100, 300:400] = w_hh[0:100, :].T
    wA[0:100, 400:500] = w_hh[100:200, :].T
    wA[0:100, 500:600] = w_hh[200:300, :].T
    # bias row (row 100): rides the ones row of the mem chunk.
    # packed r/z get b_ih+b_hh, packed n gets b_ih_n (b_hh_n lives in the
    # ghn block since it multiplies r); dense r/z identical; ghn gets b_hh_n.
    wA[100, 0:100] = b_ih[0:100] + b_hh[0:100]
    wA[100, 100:200] = b_ih[100:200] + b_hh[100:200]
    wA[100, 200:300] = b_ih[200:300]
    wA[100, 300:400] = b_ih[0:100] + b_hh[0:100]
    wA[100, 400:500] = b_ih[100:200] + b_hh[100:200]
    wA[100, 500:600] = b_hh[200:300]
    consts = np.empty((100, 4), dtype=f32)
    consts[:, 0] = b_ih[0:100] + b_hh[0:100]
    consts[:, 1] = b_ih[100:200] + b_hh[100:200]
    consts[:, 2] = b_hh[200:300]
    consts[:, 3] = b_ih[200:300]

    tw = np.asarray(time_w, dtype=f32)[:, 0]
    tb = np.asarray(time_b, dtype=f32)

    in_maps = []
    meta = []
    lu_f = last_update.astype(f32)
    tf = t.astype(f32)
    for c in range(NCORES):
        s0, s1 = starts[c], starts[c + 1]
        na = s1 - s0
        wn = (win_node[s0:s1] - c * NPC).astype(np.int64)  # local ids, ascending
        oth = other[s0:s1]
        evc = ev[s0:s1]

        act = np.zeros(NPC, dtype=bool)
        act[wn] = True
        inact = np.nonzero(~act)[0]
        perm = np.concatenate([wn, inact])

        mem_T = np.empty((MEM + 1, NPC), dtype=f32)
        mem_T[0:MEM, :] = memory[c * NPC + perm].T
        mem_T[MEM, :] = 1.0

        other_T = np.zeros((MEM, a_pad), dtype=np.float16)
        other_T[:, :na] = memory[oth].T.astype(np.float16)

        raw_T = np.zeros((RAW, a_pad), dtype=np.float16)
        raw_T[:, :na] = raw_msg[evc].T.astype(np.float16)

        # time-encoder argument: reference computes x = fl32(dt)*w + b then
        # cos(x).  We send y = range-reduced (x + pi/2) so the device's
        # ACT Sin sees [-pi, pi] and sin(y) == cos(x) to f32 precision.
        dtv = tf[evc] - lu_f[c * NPC + wn]  # f32, exact (ints < 2^24)
        x = dtv[:, None] * tw[None, :] + tb[None, :]  # f32 [na, 100]
        y = np.mod(x.astype(np.float64) + (np.pi / 2 + np.pi), 2 * np.pi) - np.pi
        arg_T = np.zeros((TDIM, a_pad), dtype=f32)
        arg_T[:, :na] = y.T.astype(f32)

        in_maps.append(
            dict(
                mem_T=mem_T,
                other_T=other_T,
                raw_T=raw_T,
                arg_T=arg_T,
                wA=wA,
                wB=wB,
                consts=consts,
            )
        )
        meta.append((na, wn, inact))

    return a_pad, d_start, in_maps, (meta, new_last_update)


def assemble(a_pad, d_start, results, meta_pack):
    meta, new_last_update = meta_pack
    new_memory = np.empty((N, MEM), dtype=np.float32)
    for c in range(NCORES):
        na, wn, inact = meta[c]
        outp = results[c]["outp_T"]  # [MEM, a_pad]
        outd = results[c]["outd_T"]  # [MEM, NPC - d_start]
        base = c * NPC
        new_memory[base + wn] = outp[:, :na].T
        new_memory[base + inact] = outd[:, na - d_start :].T
    return new_memory, new_last_update


def kernel(**inputs):
    global LAST_RESULTS
    a_pad, d_start, in_maps, meta_pack = prepare(**inputs)

    key = (a_pad, d_start)
    if key not in _PROGRAM_CACHE:
        _PROGRAM_CACHE[key] = build_program(a_pad, d_start)
    nc = _PROGRAM_CACHE[key]

    from concourse.bass_utils import run_bass_kernel_spmd

    res = run_bass_kernel_spmd(nc, in_maps, core_ids=list(range(NCORES)))
    LAST_RESULTS = res
    return assemble(a_pad, d_start, res.results, meta_pack)
